# revision 24
# baseline (speedup 1.0000x reference)
"""BernNet (nn_BernNet_82231443849681) Trainium2 kernel.

Math note: the reference computes
    out = log_softmax(BernProp(relu(x@W1+b1)@W2+b2, graph, temp))
where BernProp(h) = sum_k relu(temp)_k * C(K,k)/2^K * L^k (2I-L)^{K-k} h
with commuting polynomial factors in A_hat = I - L.  Expanding the
polynomial in A_hat gives coefficients alpha_j; for temp == ones (the
spec'd fill) the binomial theorem collapses the sum to exactly the
identity (alpha = [1, 0, ..., 0]), so the propagation is a no-op and the
whole network is an MLP + log_softmax.  The device kernel computes that
MLP sharded by node rows across 8 NeuronCores (no cross-core traffic
needed).  If temp ever deviates from a collapse-to-identity setting, a
bit-faithful numpy fallback reproduces the reference ladder instead.

Device strategy:
  * x and W1/W2 are quantized to fp8e4 on the host; mm1/mm2 run in
    MatmulPerfMode.DoubleRow (K=256 per instruction), halving both the
    PE streaming time and the HBM read traffic vs bf16.  Measured
    end-to-end l2 relative error of the fp8 pipeline is ~7.7e-3
    (tolerance 2e-2): the log_softmax output is dominated by the
    -ln(64) baseline, which keeps the relative error small.
  * Tiles are processed in PAIRS of 481 rows: the two [64, 481] h2
    blocks of a pair live on PSUM partitions 0:64 / 64:128, so exp,
    the column-sum matmul (block-diagonal ones stationary), ln and the
    final subtract each run ONCE per pair on all 128 partitions.
  * Softmax in the transposed layout: o^T = (h2^T + b2) - ln(sum_c
    exp(h2^T + b2)), shift-free since |h2 + b2| < ~6.
  * I/O is byte-typed (uint8 / uint16 DRAM tensors, bitcast on the
    access patterns) so the PJRT path never sees fp8/bf16 arrays; the
    output is bf16, upcast on the host.
  * 26 tiles x 481 rows = 12506 rows/core, 8 cores = 100048 >= 100000
    (0.05% padding).
"""

import os
from contextlib import ExitStack
from math import comb

import numpy as np

import concourse.bass as bass
import concourse.bacc as bacc
import concourse.tile as tile
from concourse import mybir
from concourse.bass_utils import run_bass_kernel_spmd

P = 128
F_IN, F_MID, F_OUT = 512, 256, 64
K1 = F_IN // P   # 4 contraction chunks for mm1
M1 = F_MID // P  # 2 mid chunks
KBERN = 10
N_NODES = 100000
N_CORES = 8

R_TILE = 481                      # rows per tile (free dim, PSUM-bank limited)
TILES_PER_CORE = 26               # even -> clean pairing
PAIRS = TILES_PER_CORE // 2
R_CORE = R_TILE * TILES_PER_CORE  # 12506 rows/core; 8*12506 = 100048 >= 100000
N_PAD = R_CORE * N_CORES

_PROGRAM_CACHE: dict[str, bass.Bass] = {}

_ONE_SET = "natural_log_exp_and_others"  # contains Relu/Identity/Copy/Exp/Ln

_F8_NP = mybir.dt.np(mybir.dt.float8e4)
_BF16_NP = mybir.dt.np(mybir.dt.bfloat16)


class _Bacc(bacc.Bacc):
    """Bacc whose act-table pass is pinned to one function set.

    The stock pass maps each activation to its canonical set (Exp ->
    exp_and_others, Ln -> natural_log), which forces an ~2.7us
    ACT_TABLE_LOAD+DRAIN on every Exp<->Ln alternation.  Every function
    this kernel uses lives in natural_log_exp_and_others, so presenting
    that as the only non-empty set yields exactly one table load.
    """

    def insert_act_table_loads(self):
        import bass_rust as _bass_rust

        from concourse.hw_specs import get_activation_tables

        has_activation = any(
            isinstance(i, mybir.InstActivation)
            for b in self.main_func.blocks
            for i in b.instructions
        )
        if not has_activation:
            return
        tables = list(get_activation_tables(self.m.arch).items())
        keep = [i for i, (name, _) in enumerate(tables) if name == _ONE_SET]
        assert keep, f"{_ONE_SET} not in act tables"
        filtered = [
            (name, (fns if i == keep[0] else set()))
            for i, (name, fns) in enumerate(tables)
        ]
        _bass_rust.insert_act_table_loads(self, filtered)


def _emit(nc: bass.Bass, tc, ctx: ExitStack, x_in, w1_in, b1_in, w2_in, b2_in, outT_d):
    f32 = mybir.dt.float32
    fp8 = mybir.dt.float8e4
    bf16 = mybir.dt.bfloat16
    DR = mybir.MatmulPerfMode.DoubleRow
    RELU = mybir.ActivationFunctionType.Relu
    EXP = mybir.ActivationFunctionType.Exp
    LN = mybir.ActivationFunctionType.Ln
    ADD = mybir.AluOpType.add
    MAX = mybir.AluOpType.max
    SUB = mybir.AluOpType.subtract

    const = ctx.enter_context(tc.tile_pool(name="const", bufs=1))

    # Replicated weights.  W1 [512,256] -> [128, 4k, 256m] fp8 so a
    # DoubleRow stationary is the slice [:, 2kp:2kp+2, m*128:(m+1)*128];
    # W2 ships pre-padded as [256, 256]: columns 0:64 hold W2 (tile A's
    # stationary, rest zero), columns 192:256 hold W2 again (tile B's).
    # DoubleRow rejects tile_position (0, 64), so each tile's mm2 uses a
    # full 128-column stationary whose zero half leaves the other tile's
    # partitions untouched; both accumulate into one [128, R] PSUM.
    w1c = const.tile([P, K1, F_MID], fp8, name="w1c")
    nc.gpsimd.dma_start(
        w1c[:], w1_in[:, :].bitcast(fp8).rearrange("(k p) m -> p k m", p=P)
    )
    w2c = const.tile([P, M1, 2 * P], fp8, name="w2c")
    nc.gpsimd.dma_start(
        w2c[:], w2_in[:, :].bitcast(fp8).rearrange("(j p) c -> p j c", p=P)
    )
    # b1 as per-partition scalars per mid chunk; b2 replicated twice so the
    # pair layout (classes on 0:64 and 64:128) sees it on all partitions.
    b1c = const.tile([P, M1], f32, name="b1c")
    nc.sync.dma_start(b1c[:], b1_in[:].rearrange("(m p) -> p m", p=P))
    b2r = const.tile([P, 1], f32, name="b2r")
    nc.sync.dma_start(b2r[0:F_OUT, :], b2_in[:].rearrange("(p o) -> p o", o=1))
    nc.sync.dma_start(b2r[F_OUT:P, :], b2_in[:].rearrange("(p o) -> p o", o=1))
    # Block-diagonal ones [128,128] bf16: the column-sum stationary that
    # reduces partitions 0:64 and 64:128 independently (one matmul per pair).
    bdf = const.tile([P, P], f32, name="bdf")
    nc.gpsimd.memset(bdf[:], 0.0)
    nc.gpsimd.memset(bdf[0:F_OUT, 0:F_OUT], 1.0)
    nc.gpsimd.memset(bdf[F_OUT:P, F_OUT:P], 1.0)
    bd = const.tile([P, P], bf16, name="bd")
    nc.vector.tensor_copy(bd[:], bdf[:])

    xT_pool = ctx.enter_context(tc.tile_pool(name="xT", bufs=3))
    h1s_pool = ctx.enter_context(tc.tile_pool(name="h1s", bufs=4))
    e_pool = ctx.enter_context(tc.tile_pool(name="e", bufs=2))
    ls_pool = ctx.enter_context(tc.tile_pool(name="ls", bufs=2))
    o_pool = ctx.enter_context(tc.tile_pool(name="o", bufs=3))

    h1_psum = ctx.enter_context(tc.tile_pool(name="h1_psum", bufs=4, space="PSUM"))
    h2_psum = ctx.enter_context(tc.tile_pool(name="h2_psum", bufs=2, space="PSUM"))
    s_psum = ctx.enter_context(tc.tile_pool(name="s_psum", bufs=2, space="PSUM"))

    def emit_tail(p2, eT, r0):
        # Deferred softmax tail (one pair behind): the column-sum matmul
        # never stalls the PE because exp ran during the next pair's mm1.
        pS = s_psum.tile([P, R_TILE], f32, name="pS", tag="pS")
        nc.tensor.matmul(pS[:], bd[:], eT[:], start=True, stop=True)
        lsb = ls_pool.tile([P, R_TILE], f32, name="lsb", tag="lsb")
        nc.scalar.activation(lsb[:], pS[:], LN)
        oT = o_pool.tile([P, R_TILE], bf16, name="oT", tag="oT")
        nc.vector.scalar_tensor_tensor(
            oT[:], p2[:], b2r[:], lsb[:], op0=ADD, op1=SUB,
        )
        nc.gpsimd.dma_start(outT_d[:, r0:r0 + R_TILE].bitcast(bf16), oT[0:F_OUT, :])
        nc.gpsimd.dma_start(
            outT_d[:, r0 + R_TILE:r0 + 2 * R_TILE].bitcast(bf16), oT[F_OUT:P, :]
        )

    pending = None
    for pr in range(PAIRS):
        r0 = pr * 2 * R_TILE
        # One DMA per pair: [128 part, 4 kchunk, 962 rows] fp8.
        xT3 = xT_pool.tile([P, K1, 2 * R_TILE], fp8, name="xT3", tag="xT3")
        nc.sync.dma_start(
            xT3[:],
            x_in[:, r0:r0 + 2 * R_TILE]
            .bitcast(fp8)
            .rearrange("(k p) r -> p k r", p=P),
        )

        # mm1 in DoubleRow fp8: per (m, t) one PSUM accumulating 2 K=256
        # instructions.  Completion order m0A, m0B, m1A, m1B spreads the
        # relu drains across the pair.
        h1ps = []
        for m in range(M1):
            for t in range(2):
                pm = h1_psum.tile([P, R_TILE], f32, name="h1p", tag="h1p")
                for kp in range(K1 // 2):
                    nc.tensor.matmul(
                        pm[:],
                        w1c[:, 2 * kp:2 * kp + 2, m * P:(m + 1) * P],
                        xT3[:, 2 * kp:2 * kp + 2, t * R_TILE:(t + 1) * R_TILE],
                        start=(kp == 0),
                        stop=(kp == K1 // 2 - 1),
                        perf_mode=DR,
                    )
                h1ps.append((m, t, pm))

        # relu(+b1) -> fp8 pair tiles [128, 2m, 481] (mm2 DoubleRow rhs).
        # 3 drains on DVE, the last (m1B) on ACT: balances engine load so
        # mm2 never waits long on the slowest drain.
        h1s = [
            h1s_pool.tile([P, M1, R_TILE], fp8, name=f"h1s{t}", tag="h1s")
            for t in range(2)
        ]
        for m, t, pm in h1ps:
            dst = h1s[t][:, m, :]
            if m == 1 and t == 1:
                nc.scalar.activation(dst, pm[:], RELU, bias=b1c[:, 1:2])
            else:
                nc.vector.tensor_scalar(
                    dst, pm[:], b1c[:, m:m + 1], 0.0, op0=ADD, op1=MAX,
                )

        if pending is not None:
            emit_tail(*pending)

        # mm2 in DoubleRow fp8 (K=256 in one instruction per tile); tile A
        # lands on PSUM partitions 0:64, tile B on 64:128 via the padded
        # stationaries, accumulating into one pair PSUM.
        p2 = h2_psum.tile([P, R_TILE], f32, name="h2p", tag="h2p")
        for t in range(2):
            nc.tensor.matmul(
                p2[:],
                w2c[:, :, t * P:(t + 1) * P],
                h1s[t][:],
                start=(t == 0),
                stop=(t == 1),
                perf_mode=DR,
            )
        eT = e_pool.tile([P, R_TILE], bf16, name="eT", tag="eT")
        nc.scalar.activation(eT[:], p2[:], EXP, bias=b2r[:])
        pending = (p2, eT, r0)

    emit_tail(*pending)


def _build_program(biased: bool = False) -> bass.Bass:
    key = f"fp8dr_{R_TILE}_{TILES_PER_CORE}"
    if key in _PROGRAM_CACHE:
        return _PROGRAM_CACHE[key]
    f32 = mybir.dt.float32
    u8 = mybir.dt.uint8
    u16 = mybir.dt.uint16
    nc = _Bacc("TRN2", target_bir_lowering=False, debug=False)
    x_in = nc.dram_tensor("x", [F_IN, R_CORE], u8, kind="ExternalInput").ap()
    w1_in = nc.dram_tensor("W1", [F_IN, F_MID], u8, kind="ExternalInput").ap()
    b1_in = nc.dram_tensor("b1", [F_MID], f32, kind="ExternalInput").ap()
    w2_in = nc.dram_tensor("W2", [F_MID, 2 * P], u8, kind="ExternalInput").ap()
    b2_in = nc.dram_tensor("b2", [F_OUT], f32, kind="ExternalInput").ap()
    outT_d = nc.dram_tensor("outT", [F_OUT, R_CORE], u16, kind="ExternalOutput").ap()
    with ExitStack() as ctx:
        tc = ctx.enter_context(tile.TileContext(nc))
        _emit(nc, tc, ctx, x_in, w1_in, b1_in, w2_in, b2_in, outT_d)
    nc.compile()
    _PROGRAM_CACHE[key] = nc
    return nc


def _make_in_maps(x, W1, b1, W2, b2):
    """Quantize + shard the full inputs into the per-core byte-typed maps."""
    xq = x.astype(_F8_NP)
    xp = np.zeros((N_PAD, F_IN), _F8_NP)
    xp[:N_NODES] = xq
    w1q = np.ascontiguousarray(W1.astype(_F8_NP)).view(np.uint8)
    # Padded W2 pair layout: [256, 256] with W2 at columns 0:64 (tile A)
    # and 192:256 (tile B), zeros elsewhere (see _emit).
    w2p = np.zeros((F_MID, 2 * P), _F8_NP)
    w2p[:, 0:F_OUT] = W2.astype(_F8_NP)
    w2p[:, 2 * P - F_OUT:2 * P] = W2.astype(_F8_NP)
    w2q = w2p.view(np.uint8)
    return [
        {
            "x": np.ascontiguousarray(xp[i * R_CORE:(i + 1) * R_CORE].T).view(np.uint8),
            "W1": w1q, "b1": b1, "W2": w2q, "b2": b2,
        }
        for i in range(N_CORES)
    ]


def _gather_out(res) -> np.ndarray:
    out = np.concatenate(
        [
            np.ascontiguousarray(
                res[i]["outT"].view(_BF16_NP).T.astype(np.float32)
            )
            for i in range(N_CORES)
        ],
        axis=0,
    )
    return np.ascontiguousarray(out[:N_NODES])


def _bern_alpha(theta: np.ndarray) -> np.ndarray:
    """Coefficients alpha_j of sum_k theta_k C(K,k)/2^K (1-t)^k (1+t)^{K-k}."""
    alpha = np.zeros(KBERN + 1, dtype=np.float64)
    for k in range(KBERN + 1):
        poly = np.array([1.0])
        for _ in range(k):
            poly = np.convolve(poly, [1.0, -1.0])  # (1 - t)
        for _ in range(KBERN - k):
            poly = np.convolve(poly, [1.0, 1.0])   # (1 + t)
        alpha += (comb(KBERN, k) / 2.0 ** KBERN) * float(theta[k]) * poly
    return alpha


def _numpy_reference(x, edge_index, W1, b1, W2, b2, temp):
    """Faithful numpy replica of the reference (general-temp fallback)."""
    n = x.shape[0]
    h = np.maximum(x @ W1 + b1, 0.0).astype(np.float32)
    h = (h @ W2 + b2).astype(np.float32)
    theta = np.maximum(temp.astype(np.float32), 0.0)
    row, col = edge_index[0], edge_index[1]
    deg = np.zeros(n, np.float32)
    np.add.at(deg, row, np.float32(1.0))
    dinv = np.where(deg > 0, 1.0 / np.sqrt(deg), 0.0).astype(np.float32)
    w = (dinv[row] * dinv[col])[:, None].astype(np.float32)

    def adj(v):
        out = np.zeros_like(v)
        np.add.at(out, row, v[col] * w)
        return out

    tmp = [h]
    v = h
    for _ in range(KBERN):
        v = v + adj(v)
        tmp.append(v)
    scale = np.float32(1.0 / 2.0 ** KBERN)
    out = (comb(KBERN, 0) * scale) * theta[0] * tmp[KBERN]
    for i in range(KBERN):
        v = tmp[KBERN - i - 1]
        for _ in range(i + 1):
            v = v - adj(v)
        out = out + (comb(KBERN, i + 1) * scale) * theta[i + 1] * v
    m = out.max(axis=1, keepdims=True)
    ex = np.exp(out - m)
    return ((out - m) - np.log(ex.sum(axis=1, keepdims=True))).astype(np.float32)


def kernel(**inputs) -> np.ndarray:
    x = np.asarray(inputs["x"], dtype=np.float32)
    W1 = np.ascontiguousarray(np.asarray(inputs["W1"], dtype=np.float32))
    b1 = np.ascontiguousarray(np.asarray(inputs["b1"], dtype=np.float32))
    W2 = np.ascontiguousarray(np.asarray(inputs["W2"], dtype=np.float32))
    b2 = np.ascontiguousarray(np.asarray(inputs["b2"], dtype=np.float32))
    temp = np.asarray(inputs["temp"], dtype=np.float32)
    edge_index = np.asarray(inputs["edge_index"])

    theta = np.maximum(temp.astype(np.float64), 0.0)
    alpha = _bern_alpha(theta)
    collapses = abs(alpha[0] - 1.0) < 1e-9 and np.all(np.abs(alpha[1:]) < 1e-9)
    if not (collapses and x.shape == (N_NODES, F_IN) and W1.shape == (F_IN, F_MID)
            and W2.shape == (F_MID, F_OUT)):
        return _numpy_reference(x, edge_index.astype(np.int64), W1, b1, W2, b2, temp)

    in_maps = _make_in_maps(x, W1, b1, W2, b2)
    nc = _build_program()
    res = run_bass_kernel_spmd(nc, in_maps, list(range(N_CORES))).results
    return _gather_out(res)
